# revision 12
# baseline (speedup 1.0000x reference)
"""GQA attention kernel for 8 Trainium2 NeuronCores (v2).

Sharding: core c = 4*b + h handles batch b (of 2) and kv-head h (of 4),
i.e. one kv head + its 4 grouped query heads. Each core computes its head
group's partial contribution to the output projection; the host sums the
4 partials per batch. No collectives.

v2 changes vs v1 (461us):
  - all inputs bf16 (halves DMA, full-rate matmuls), BAND=512 projections
  - softmax denominator via vector accumulation of P tiles + gpsimd
    partition_all_reduce -- no PE den matmuls (-18% PE stream), no slow
    [1,512] reciprocal (3.3us each)
  - attention inner loop software-pipelined: scores(sk+1) issued between
    PV(sk) matmuls so the exp latency never stalls the PE
  - QTILE=1024 (fewer, longer instruction groups), out-projection of
    qtile 0 interleaved between attention g-blocks of qtile 1
  - output partials in bf16 (halves output DMA)

Device math per core (S=2048, H=2048, d=128):
  QT_g = (x @ Wq_g + bq_g)^T          [d, S]   g=0..3   (bf16 matmuls)
  KT   = (x @ Wk_h)^T                 [d, S]            (bk cancels in softmax)
  V    = x @ Wv_h                     [S, d]   (V^T then PE-transposed)
  S^T  = KT^T-blocks @ QT             [Sk, Sq]
  P^T  = exp(SCALE * S^T)             (bf16, no max-subtraction: |s| <~ 5)
  y^T  = V^T-blocks.T @ P^T (PSUM accum); den = partition_all_reduce(sum P^T)
  yT  := y^T * recip(den)
  out += yT_g^T @ Wo_g                [S, H]  partial, bf16 to HBM
Host: out[b] = sum_h partial + (bv_rep @ Wo + bo).
"""

import numpy as np
import ml_dtypes

B = 2
S = 2048
HIDDEN = 2048
NKV = 4
GROUP = 4
D = 128
SCALE = D ** -0.5

BAND = 512            # S-columns per projection band
NBAND = S // BAND     # 4
NCH = HIDDEN // 128   # 16 contraction chunks
QT2 = 1024            # queries per attention tile
NQT2 = S // QT2       # 2
NSK = S // 128        # 16 key tiles

_CACHE = {}
LAST_RESULTS = None
TRACE = False
TMPDIR = None


def _build():
    import concourse.bass as bass
    import concourse.bacc as bacc
    import concourse.mybir as mybir
    import concourse.tile as tile
    import concourse.bass_isa as bass_isa
    from concourse.masks import make_identity

    f32 = mybir.dt.float32
    bf16 = mybir.dt.bfloat16
    EXP = mybir.ActivationFunctionType.Exp
    IDENT = mybir.ActivationFunctionType.Identity
    COPY = mybir.ActivationFunctionType.Copy

    nc = bacc.Bacc(trn_type="TRN2", target_bir_lowering=False, debug=False)

    xT = nc.dram_tensor("xT", [NBAND, 128, NCH, BAND], bf16, kind="ExternalInput").ap()
    wq = nc.dram_tensor("wq", [GROUP, 128, NCH, 128], bf16, kind="ExternalInput").ap()
    wk = nc.dram_tensor("wk", [128, NCH, 128], bf16, kind="ExternalInput").ap()
    wv = nc.dram_tensor("wv", [128, NCH, 128], bf16, kind="ExternalInput").ap()
    wo = nc.dram_tensor("wo", [GROUP, 128, HIDDEN], bf16, kind="ExternalInput").ap()
    bq = nc.dram_tensor("bq", [128, GROUP], f32, kind="ExternalInput").ap()
    out = nc.dram_tensor("out", [S, HIDDEN], bf16, kind="ExternalOutput").ap()

    with tile.TileContext(nc) as tc:
        with (
            tc.tile_pool(name="const", bufs=1) as constp,
            tc.tile_pool(name="wts", bufs=1) as wtsp,
            tc.tile_pool(name="xb", bufs=2) as xbp,
            tc.tile_pool(name="qkv", bufs=1) as qkvp,
            tc.tile_pool(name="ptbuf", bufs=4) as ptp,
            tc.tile_pool(name="accb", bufs=2) as accp,
            tc.tile_pool(name="dens", bufs=2) as densp,
            tc.tile_pool(name="ytbuf", bufs=8) as ytp,
            tc.tile_pool(name="outbuf", bufs=2) as outp,
        ):
            # ---- DMAs in strict consumption order; each dma_start costs ~2us
            # of descriptor overhead, so the first K-proj inputs go first ----
            wk_t = wtsp.tile([128, NCH, 128], bf16, name="wk_t")
            nc.sync.dma_start(out=wk_t[:, 0:4, :], in_=wk[:, 0:4, :])
            bands = [None] * NBAND
            bands[0] = xbp.tile([128, NCH, BAND], bf16, name="band", tag="band")
            nc.sync.dma_start(out=bands[0][:, 0:4, :], in_=xT[0, :, 0:4, :])
            nc.sync.dma_start(out=wk_t[:, 4:16, :], in_=wk[:, 4:16, :])
            nc.sync.dma_start(out=bands[0][:, 4:8, :], in_=xT[0, :, 4:8, :])
            nc.sync.dma_start(out=bands[0][:, 8:16, :], in_=xT[0, :, 8:16, :])
            bq_t = constp.tile([128, GROUP], f32, name="bq_t")
            nc.sync.dma_start(out=bq_t[:, :], in_=bq)
            ident = constp.tile([128, 128], f32, name="ident")
            make_identity(nc, ident[:, :])

            wv_t = wtsp.tile([128, NCH, 128], bf16, name="wv_t")
            nc.sync.dma_start(out=wv_t[:, :, :], in_=wv)
            wq_t = []
            for g in range(GROUP):
                t = wtsp.tile([128, NCH, 128], bf16, name=f"wq_t{g}", tag=f"wq{g}")
                nc.sync.dma_start(out=t[:, :, :], in_=wq[g])
                wq_t.append(t)

            # ---- persistent activations ----
            qt_t = []
            for g in range(GROUP):
                t = qkvp.tile([128, S], bf16, name=f"qt{g}", tag=f"qt{g}")
                qt_t.append(t)
            kt_t = qkvp.tile([128, S], bf16, name="kt_t")
            v_t = qkvp.tile([128, NSK * 128], bf16, name="v_t")
            vt_f = qkvp.tile([128, S], f32, name="vt_f")

            # =============== phase 1: projections ===============
            with tc.tile_pool(name="psA", bufs=1, space="PSUM") as psA:
                for b in range(NBAND):
                    if bands[b] is None:
                        bands[b] = xbp.tile(
                            [128, NCH, BAND], bf16, name="band", tag="band"
                        )
                        nc.sync.dma_start(out=bands[b][:, :, :], in_=xT[b])
                    band = bands[b]
                    bsl = slice(b * BAND, (b + 1) * BAND)

                    # K^T accumulation
                    pk = psA.tile([128, BAND], f32, name="pk", tag="pacc", bufs=4)
                    for c in range(NCH):
                        nc.tensor.matmul(
                            out=pk[:, :],
                            lhsT=wk_t[:, c, :],
                            rhs=band[:, c, :],
                            start=(c == 0), stop=(c == NCH - 1),
                        )
                    nc.scalar.activation(kt_t[:, bsl], pk[:, :], COPY)

                    # V^T accumulation (f32, transposed to V per 128-block later)
                    pv = psA.tile([128, BAND], f32, name="pv", tag="pacc", bufs=4)
                    for c in range(NCH):
                        nc.tensor.matmul(
                            out=pv[:, :],
                            lhsT=wv_t[:, c, :],
                            rhs=band[:, c, :],
                            start=(c == 0), stop=(c == NCH - 1),
                        )
                    nc.scalar.activation(vt_f[:, bsl], pv[:, :], COPY)

                    # Q^T per local head
                    for g in range(GROUP):
                        pq = psA.tile([128, BAND], f32, name="pq", tag="pacc", bufs=4)
                        for c in range(NCH):
                            nc.tensor.matmul(
                                out=pq[:, :],
                                lhsT=wq_t[g][:, c, :],
                                rhs=band[:, c, :],
                                start=(c == 0), stop=(c == NCH - 1),
                            )
                        nc.scalar.activation(
                            qt_t[g][:, bsl], pq[:, :], IDENT,
                            bias=bq_t[:, g:g + 1],
                        )

                    # transpose V^T band -> V (4 sk-tiles per band)
                    for t in range(BAND // 128):
                        sk = b * (BAND // 128) + t
                        pt = psA.tile([128, 128], f32, name="ptr", tag="pacc", bufs=4)
                        nc.tensor.transpose(
                            pt[:, :], vt_f[:, sk * 128:(sk + 1) * 128], ident[:, :]
                        )
                        nc.scalar.activation(
                            v_t[:, sk * 128:(sk + 1) * 128], pt[:, :], COPY
                        )

            # wo loads (needed only by out-projection, off the critical path)
            wo_t = []
            for g in range(GROUP):
                t = wtsp.tile([128, HIDDEN], bf16, name=f"wo_t{g}", tag=f"wo{g}")
                nc.sync.dma_start(out=t[:, :], in_=wo[g])
                wo_t.append(t)

            # =============== phase 2+3: attention + out-projection ===============
            # PSUM budget (8 banks): tag "pp" [128,1024] bufs=2 (4 banks),
            # shared by score pairs and out-proj pairs (never live at once);
            # tag "py" [128,1024] bufs=2 (4 banks).
            with tc.tile_pool(name="psB", bufs=1, space="PSUM") as psB:
                yt_all = {}

                def attn_block(qt2, g, pending_fin=None):
                    """Attention for one head g over query tile qt2 (1024 q).

                    Inner loop software-pipelined: scores for sk+1 are issued
                    between the PV matmuls of sk so the PE never waits on exp.
                    One [128,1024] exp per sk (half the scalar-engine overhead
                    of two 512-wide ones).

                    The normalize tail (recip + y*recip) of the PREVIOUS block
                    is emitted mid-loop (pending_fin): the 6.7us gpsimd
                    all-reduce would otherwise stall the whole vector queue --
                    and with it this block's acc adds, pt recycling, and exp.
                    """
                    q0 = qt2 * QT2
                    py = psB.tile([128, QT2], f32, name="py", tag="py", bufs=2)
                    acc = accp.tile([128, QT2], bf16, name="acc", tag="acc")
                    pts = [None] * NSK
                    pss = [None] * NSK

                    def scores(sk, h):
                        if h == 0:
                            pss[sk] = psB.tile(
                                [128, QT2], f32, name="ps", tag="pp", bufs=2
                            )
                        nc.tensor.matmul(
                            out=pss[sk][:, h * 512:(h + 1) * 512],
                            lhsT=kt_t[:, sk * 128:(sk + 1) * 128],
                            rhs=qt_t[g][:, q0 + h * 512: q0 + (h + 1) * 512],
                            start=True, stop=True,
                        )
                        if h == 1:
                            pts[sk] = ptp.tile([128, QT2], bf16, name="pt", tag="pt")
                            nc.scalar.activation(
                                pts[sk][:, :], pss[sk][:, :], EXP, scale=SCALE,
                            )

                    scores(0, 0)
                    scores(0, 1)
                    for sk in range(NSK):
                        if sk == 6 and pending_fin is not None:
                            pending_fin()
                            pending_fin = None
                        if sk + 1 < NSK:
                            scores(sk + 1, 0)
                            scores(sk + 1, 1)
                        for h in range(2):
                            nc.tensor.matmul(
                                out=py[:, h * 512:(h + 1) * 512],
                                lhsT=v_t[:, sk * 128:(sk + 1) * 128],
                                rhs=pts[sk][:, h * 512:(h + 1) * 512],
                                start=(sk == 0), stop=(sk == NSK - 1),
                            )
                        if sk == 0:
                            # init copy on scalar: vector COPY is slow (2.6us)
                            # and would delay the add chain
                            nc.scalar.activation(acc[:, :], pts[sk][:, :], COPY)
                        else:
                            nc.vector.tensor_add(acc[:, :], acc[:, :], pts[sk][:, :])

                    # denominator: all-partition sum of acc (bf16 in, f32 out)
                    # on gpsimd now; recip + scale deferred to the next block
                    bden = densp.tile([128, QT2], f32, name="bden", tag="bden")
                    nc.gpsimd.partition_all_reduce(
                        bden[:, :], acc[:, :], channels=128,
                        reduce_op=bass_isa.ReduceOp.add,
                    )
                    yt = ytp.tile([128, QT2], bf16, name="yt", tag="yt")
                    yt_all[(qt2, g)] = yt

                    def fin():
                        brecip = densp.tile(
                            [128, QT2], f32, name="brecip", tag="brecip"
                        )
                        nc.vector.reciprocal_approx_fast(brecip[:, :], bden[:, :])
                        nc.vector.tensor_mul(yt[:, :], py[:, :], brecip[:, :])

                    return fin

                def outproj_iblocks(qt2, iblocks):
                    for i in iblocks:
                        outs = outp.tile([128, HIDDEN], bf16, name="outs", tag="outs")
                        for jp in range(2):
                            po = psB.tile([128, QT2], f32, name="po", tag="pp", bufs=2)
                            for jj in range(2):
                                j = jp * 2 + jj
                                for g in range(GROUP):
                                    nc.tensor.matmul(
                                        out=po[:, jj * 512:(jj + 1) * 512],
                                        lhsT=yt_all[(qt2, g)][:, i * 128:(i + 1) * 128],
                                        rhs=wo_t[g][:, j * 512:(j + 1) * 512],
                                        start=(g == 0), stop=(g == GROUP - 1),
                                    )
                            # drains on scalar only: on vector they queue behind
                            # the normalize chain (all_reduce -> recip -> mul)
                            # and stall the next po allocation for ~6us
                            nc.scalar.activation(
                                outs[:, jp * 1024:(jp + 1) * 1024], po[:, :], COPY
                            )
                        r0 = qt2 * QT2 + i * 128
                        nc.sync.dma_start(out=out[r0:r0 + 128, :], in_=outs[:, :])

                fin = None
                for g in range(GROUP):
                    fin = attn_block(0, g, fin)
                # out-proj of qtile 0 spread between qtile-1 attention blocks
                for g in range(GROUP):
                    fin = attn_block(1, g, fin)
                    outproj_iblocks(0, [2 * g, 2 * g + 1])
                fin()
                outproj_iblocks(1, list(range(8)))

    nc.finalize()
    return nc


def _get_nc():
    if "nc" not in _CACHE:
        _CACHE["nc"] = _build()
    return _CACHE["nc"]


def kernel(x, Wq, bq, Wk, bk, Wv, bv, Wo, bo):
    global LAST_RESULTS
    from concourse.bass_utils import run_bass_kernel_spmd

    bf = ml_dtypes.bfloat16
    x = np.asarray(x, np.float32)
    Wq = np.asarray(Wq, np.float32)
    Wk = np.asarray(Wk, np.float32)
    Wv = np.asarray(Wv, np.float32)
    Wo = np.asarray(Wo, np.float32)
    bq = np.asarray(bq, np.float32)
    bv = np.asarray(bv, np.float32)
    bo = np.asarray(bo, np.float32)

    nc = _get_nc()

    in_maps = []
    for c in range(8):
        b, h = divmod(c, NKV)
        xT = x[b].T  # [HIDDEN, S]
        xTh = np.ascontiguousarray(
            xT.reshape(NCH, 128, NBAND, BAND).transpose(2, 1, 0, 3)
        ).astype(bf)
        # wq[g]: [128, NCH, 128] per local head
        wqh = np.ascontiguousarray(
            Wq[:, h * 512:(h + 1) * 512]
            .reshape(NCH, 128, GROUP, 128).transpose(2, 1, 0, 3)
        ).astype(bf)
        wkh = np.ascontiguousarray(
            Wk[:, h * 128:(h + 1) * 128].reshape(NCH, 128, 128).transpose(1, 0, 2)
        ).astype(bf)
        wvh = np.ascontiguousarray(
            Wv[:, h * 128:(h + 1) * 128].reshape(NCH, 128, 128).transpose(1, 0, 2)
        ).astype(bf)
        woh = np.ascontiguousarray(
            Wo[h * 512:(h + 1) * 512, :].reshape(GROUP, 128, HIDDEN)
        ).astype(bf)
        bqh = np.ascontiguousarray(
            bq[h * 512:(h + 1) * 512].reshape(GROUP, 128).T
        )
        in_maps.append({
            "xT": xTh, "wq": wqh, "wk": wkh, "wv": wvh, "wo": woh,
            "bq": bqh,
        })

    res = run_bass_kernel_spmd(
        nc, in_maps, list(range(8)), trace=TRACE, tmpdir=TMPDIR
    )
    LAST_RESULTS = res

    # host-side constant bias: (bv repeated per head group) @ Wo + bo
    bv_rep = np.broadcast_to(
        bv.reshape(NKV, 1, D), (NKV, GROUP, D)
    ).reshape(HIDDEN)
    bias_row = bv_rep @ Wo + bo  # [HIDDEN]

    out = np.empty((B, S, HIDDEN), np.float32)
    for b in range(B):
        acc = res.results[b * NKV + 0]["out"].astype(np.float32)
        for h in range(1, NKV):
            acc = acc + res.results[b * NKV + h]["out"].astype(np.float32)
        out[b] = acc + bias_row
    return out


# revision 17
# speedup vs baseline: 1.0371x; 1.0371x over previous
"""GQA attention kernel for 8 Trainium2 NeuronCores (v2).

Sharding: core c = 4*b + h handles batch b (of 2) and kv-head h (of 4),
i.e. one kv head + its 4 grouped query heads. Each core computes its head
group's partial contribution to the output projection; the host sums the
4 partials per batch. No collectives.

v2 changes vs v1 (461us):
  - all inputs bf16 (halves DMA, full-rate matmuls), BAND=512 projections
  - softmax denominator via vector accumulation of P tiles + gpsimd
    partition_all_reduce -- no PE den matmuls (-18% PE stream), no slow
    [1,512] reciprocal (3.3us each)
  - attention inner loop software-pipelined: scores(sk+1) issued between
    PV(sk) matmuls so the exp latency never stalls the PE
  - QTILE=1024 (fewer, longer instruction groups), out-projection of
    qtile 0 interleaved between attention g-blocks of qtile 1
  - output partials in bf16 (halves output DMA)

Device math per core (S=2048, H=2048, d=128):
  QT_g = (x @ Wq_g + bq_g)^T          [d, S]   g=0..3   (bf16 matmuls)
  KT   = (x @ Wk_h)^T                 [d, S]            (bk cancels in softmax)
  V    = x @ Wv_h                     [S, d]   (V^T then PE-transposed)
  S^T  = KT^T-blocks @ QT             [Sk, Sq]
  P^T  = exp(SCALE * S^T)             (bf16, no max-subtraction: |s| <~ 5)
  y^T  = V^T-blocks.T @ P^T (PSUM accum); den = partition_all_reduce(sum P^T)
  yT  := y^T * recip(den)
  out += yT_g^T @ Wo_g                [S, H]  partial, bf16 to HBM
Host: out[b] = sum_h partial + (bv_rep @ Wo + bo).
"""

import numpy as np
import ml_dtypes

B = 2
S = 2048
HIDDEN = 2048
NKV = 4
GROUP = 4
D = 128
SCALE = D ** -0.5

BAND = 512            # S-columns per projection band
NBAND = S // BAND     # 4
NCH = HIDDEN // 128   # 16 contraction chunks
QT2 = 1024            # queries per attention tile
NQT2 = S // QT2       # 2
NSK = S // 128        # 16 key tiles

_CACHE = {}
LAST_RESULTS = None
TRACE = False
TMPDIR = None


def _build():
    import concourse.bass as bass
    import concourse.bacc as bacc
    import concourse.mybir as mybir
    import concourse.tile as tile
    import concourse.bass_isa as bass_isa
    from concourse.masks import make_identity

    f32 = mybir.dt.float32
    bf16 = mybir.dt.bfloat16
    EXP = mybir.ActivationFunctionType.Exp
    IDENT = mybir.ActivationFunctionType.Identity
    COPY = mybir.ActivationFunctionType.Copy

    nc = bacc.Bacc(trn_type="TRN2", target_bir_lowering=False, debug=False)

    xT = nc.dram_tensor("xT", [NBAND, 128, NCH, BAND], bf16, kind="ExternalInput").ap()
    wq = nc.dram_tensor("wq", [GROUP, 128, NCH, 128], bf16, kind="ExternalInput").ap()
    wk = nc.dram_tensor("wk", [128, NCH, 128], bf16, kind="ExternalInput").ap()
    wv = nc.dram_tensor("wv", [128, NCH, 128], bf16, kind="ExternalInput").ap()
    wo = nc.dram_tensor("wo", [GROUP, 128, HIDDEN], bf16, kind="ExternalInput").ap()
    bq = nc.dram_tensor("bq", [128, GROUP], f32, kind="ExternalInput").ap()
    out = nc.dram_tensor("out", [S, HIDDEN], bf16, kind="ExternalOutput").ap()

    with tile.TileContext(nc) as tc:
        with (
            tc.tile_pool(name="const", bufs=1) as constp,
            tc.tile_pool(name="wts", bufs=1) as wtsp,
            tc.tile_pool(name="xb", bufs=2) as xbp,
            tc.tile_pool(name="qkv", bufs=1) as qkvp,
            tc.tile_pool(name="ptbuf", bufs=4) as ptp,
            tc.tile_pool(name="accb", bufs=2) as accp,
            tc.tile_pool(name="dens", bufs=2) as densp,
            tc.tile_pool(name="ytbuf", bufs=8) as ytp,
            tc.tile_pool(name="outbuf", bufs=2) as outp,
        ):
            # ---- DMAs in strict consumption order; each dma_start costs ~2us
            # of descriptor overhead, so the first K-proj inputs go first ----
            wk_t = wtsp.tile([128, NCH, 128], bf16, name="wk_t")
            nc.sync.dma_start(out=wk_t[:, 0:4, :], in_=wk[:, 0:4, :])
            bands = [None] * NBAND
            bands[0] = xbp.tile([128, NCH, BAND], bf16, name="band", tag="band")
            nc.sync.dma_start(out=bands[0][:, 0:4, :], in_=xT[0, :, 0:4, :])
            nc.sync.dma_start(out=wk_t[:, 4:16, :], in_=wk[:, 4:16, :])
            nc.sync.dma_start(out=bands[0][:, 4:8, :], in_=xT[0, :, 4:8, :])
            nc.sync.dma_start(out=bands[0][:, 8:16, :], in_=xT[0, :, 8:16, :])
            bq_t = constp.tile([128, GROUP], f32, name="bq_t")
            nc.sync.dma_start(out=bq_t[:, :], in_=bq)
            ident = constp.tile([128, 128], f32, name="ident")
            make_identity(nc, ident[:, :])

            wv_t = wtsp.tile([128, NCH, 128], bf16, name="wv_t")
            nc.sync.dma_start(out=wv_t[:, :, :], in_=wv)
            wq_t = []
            for g in range(GROUP):
                t = wtsp.tile([128, NCH, 128], bf16, name=f"wq_t{g}", tag=f"wq{g}")
                nc.sync.dma_start(out=t[:, :, :], in_=wq[g])
                wq_t.append(t)

            # ---- persistent activations ----
            qt_t = []
            for g in range(GROUP):
                t = qkvp.tile([128, S], bf16, name=f"qt{g}", tag=f"qt{g}")
                qt_t.append(t)
            kt_t = qkvp.tile([128, S], bf16, name="kt_t")
            v_t = qkvp.tile([128, NSK * 128], bf16, name="v_t")
            vt_f = qkvp.tile([128, S], f32, name="vt_f")

            # =============== phase 1: projections ===============
            with tc.tile_pool(name="psA", bufs=1, space="PSUM") as psA:
                for b in range(NBAND):
                    if bands[b] is None:
                        bands[b] = xbp.tile(
                            [128, NCH, BAND], bf16, name="band", tag="band"
                        )
                        nc.sync.dma_start(out=bands[b][:, :, :], in_=xT[b])
                    band = bands[b]
                    bsl = slice(b * BAND, (b + 1) * BAND)

                    # K^T accumulation
                    pk = psA.tile([128, BAND], f32, name="pk", tag="pacc", bufs=4)
                    for c in range(NCH):
                        nc.tensor.matmul(
                            out=pk[:, :],
                            lhsT=wk_t[:, c, :],
                            rhs=band[:, c, :],
                            start=(c == 0), stop=(c == NCH - 1),
                        )
                    nc.scalar.activation(kt_t[:, bsl], pk[:, :], COPY)

                    # V^T accumulation (f32, transposed to V per 128-block later)
                    pv = psA.tile([128, BAND], f32, name="pv", tag="pacc", bufs=4)
                    for c in range(NCH):
                        nc.tensor.matmul(
                            out=pv[:, :],
                            lhsT=wv_t[:, c, :],
                            rhs=band[:, c, :],
                            start=(c == 0), stop=(c == NCH - 1),
                        )
                    nc.scalar.activation(vt_f[:, bsl], pv[:, :], COPY)

                    # Q^T per local head -- only for the first half of the
                    # queries (bands 0-1); bands 2-3 Q projections are
                    # deferred into the qtile-0 attention blocks, which are
                    # otherwise scalar(exp)-bound with PE slack
                    if b < 2:
                        for g in range(GROUP):
                            pq = psA.tile(
                                [128, BAND], f32, name="pq", tag="pacc", bufs=4
                            )
                            for c in range(NCH):
                                nc.tensor.matmul(
                                    out=pq[:, :],
                                    lhsT=wq_t[g][:, c, :],
                                    rhs=band[:, c, :],
                                    start=(c == 0), stop=(c == NCH - 1),
                                )
                            nc.scalar.activation(
                                qt_t[g][:, bsl], pq[:, :], IDENT,
                                bias=bq_t[:, g:g + 1],
                            )

                    # transpose V^T band -> V (4 sk-tiles per band)
                    for t in range(BAND // 128):
                        sk = b * (BAND // 128) + t
                        pt = psA.tile([128, 128], f32, name="ptr", tag="pacc", bufs=4)
                        nc.tensor.transpose(
                            pt[:, :], vt_f[:, sk * 128:(sk + 1) * 128], ident[:, :]
                        )
                        nc.scalar.activation(
                            v_t[:, sk * 128:(sk + 1) * 128], pt[:, :], COPY
                        )

            # wo loads (needed only by out-projection, off the critical path)
            wo_t = []
            for g in range(GROUP):
                t = wtsp.tile([128, HIDDEN], bf16, name=f"wo_t{g}", tag=f"wo{g}")
                nc.sync.dma_start(out=t[:, :], in_=wo[g])
                wo_t.append(t)

            # =============== phase 2+3: attention + out-projection ===============
            # PSUM budget (8 banks): tag "pp" [128,1024] bufs=2 (4 banks),
            # shared by score pairs and out-proj pairs (never live at once);
            # tag "py" [128,1024] bufs=2 (4 banks).
            with tc.tile_pool(name="psB", bufs=1, space="PSUM") as psB:
                yt_all = {}

                def qproj_band(g, b):
                    """Deferred Q projection of head g, band b (2 or 3)."""
                    pq = psB.tile([128, QT2], f32, name="pqd", tag="pp", bufs=2)
                    for c in range(NCH):
                        nc.tensor.matmul(
                            out=pq[:, 0:BAND],
                            lhsT=wq_t[g][:, c, :],
                            rhs=bands[b][:, c, :],
                            start=(c == 0), stop=(c == NCH - 1),
                        )
                    nc.scalar.activation(
                        qt_t[g][:, b * BAND:(b + 1) * BAND], pq[:, 0:BAND],
                        IDENT, bias=bq_t[:, g:g + 1],
                    )

                def attn_block(qt2, g, pending_fin=None, pads=()):
                    """Attention for one head g over query tile qt2 (1024 q).

                    Inner loop software-pipelined: scores for sk+1 are issued
                    between the PV matmuls of sk so the PE never waits on exp.
                    One [128,1024] exp per sk (half the scalar-engine overhead
                    of two 512-wide ones).

                    The normalize tail (recip + y*recip) of the PREVIOUS block
                    is emitted mid-loop (pending_fin): the 6.7us gpsimd
                    all-reduce would otherwise stall the whole vector queue --
                    and with it this block's acc adds, pt recycling, and exp.
                    """
                    q0 = qt2 * QT2
                    py = psB.tile([128, QT2], f32, name="py", tag="py", bufs=2)
                    acc = accp.tile([128, QT2], bf16, name="acc", tag="acc")
                    pts = [None] * NSK
                    pss = [None] * NSK

                    def scores(sk, h):
                        if h == 0:
                            pss[sk] = psB.tile(
                                [128, QT2], f32, name="ps", tag="pp", bufs=2
                            )
                        nc.tensor.matmul(
                            out=pss[sk][:, h * 512:(h + 1) * 512],
                            lhsT=kt_t[:, sk * 128:(sk + 1) * 128],
                            rhs=qt_t[g][:, q0 + h * 512: q0 + (h + 1) * 512],
                            start=True, stop=True,
                        )
                        if h == 1:
                            pts[sk] = ptp.tile([128, QT2], bf16, name="pt", tag="pt")
                            nc.scalar.activation(
                                pts[sk][:, :], pss[sk][:, :], EXP, scale=SCALE,
                            )

                    scores(0, 0)
                    scores(0, 1)
                    for sk in range(NSK):
                        if sk == 3 and len(pads) > 0:
                            pads[0]()
                        if sk == 6 and pending_fin is not None:
                            pending_fin()
                            pending_fin = None
                        if sk == 9 and len(pads) > 1:
                            pads[1]()
                        if sk + 1 < NSK:
                            scores(sk + 1, 0)
                            scores(sk + 1, 1)
                        for h in range(2):
                            nc.tensor.matmul(
                                out=py[:, h * 512:(h + 1) * 512],
                                lhsT=v_t[:, sk * 128:(sk + 1) * 128],
                                rhs=pts[sk][:, h * 512:(h + 1) * 512],
                                start=(sk == 0), stop=(sk == NSK - 1),
                            )
                        if sk == 0:
                            # init copy on scalar: vector COPY is slow (2.6us)
                            # and would delay the add chain
                            nc.scalar.activation(acc[:, :], pts[sk][:, :], COPY)
                        else:
                            nc.vector.tensor_add(acc[:, :], acc[:, :], pts[sk][:, :])

                    # denominator: all-partition sum of acc (bf16 in, f32 out)
                    # on gpsimd now; recip + scale deferred to the next block
                    bden = densp.tile([128, QT2], f32, name="bden", tag="bden")
                    nc.gpsimd.partition_all_reduce(
                        bden[:, :], acc[:, :], channels=128,
                        reduce_op=bass_isa.ReduceOp.add,
                    )
                    yt = ytp.tile([128, QT2], bf16, name="yt", tag="yt")
                    yt_all[(qt2, g)] = yt

                    def fin():
                        brecip = densp.tile(
                            [128, QT2], f32, name="brecip", tag="brecip"
                        )
                        nc.vector.reciprocal_approx_fast(brecip[:, :], bden[:, :])
                        nc.vector.tensor_mul(yt[:, :], py[:, :], brecip[:, :])

                    return fin

                def outproj_iblocks(qt2, iblocks):
                    for i in iblocks:
                        outs = outp.tile([128, HIDDEN], bf16, name="outs", tag="outs")
                        for jp in range(2):
                            po = psB.tile([128, QT2], f32, name="po", tag="pp", bufs=2)
                            for jj in range(2):
                                j = jp * 2 + jj
                                for g in range(GROUP):
                                    nc.tensor.matmul(
                                        out=po[:, jj * 512:(jj + 1) * 512],
                                        lhsT=yt_all[(qt2, g)][:, i * 128:(i + 1) * 128],
                                        rhs=wo_t[g][:, j * 512:(j + 1) * 512],
                                        start=(g == 0), stop=(g == GROUP - 1),
                                    )
                            # drains on scalar only: on vector they queue behind
                            # the normalize chain (all_reduce -> recip -> mul)
                            # and stall the next po allocation for ~6us
                            nc.scalar.activation(
                                outs[:, jp * 1024:(jp + 1) * 1024], po[:, :], COPY
                            )
                        r0 = qt2 * QT2 + i * 128
                        nc.sync.dma_start(out=out[r0:r0 + 128, :], in_=outs[:, :])

                fin = None
                for g in range(GROUP):
                    # deferred Q projections pad the scalar-bound qt0 blocks
                    fin = attn_block(
                        0, g, fin,
                        pads=(
                            lambda g=g: qproj_band(g, 2),
                            lambda g=g: qproj_band(g, 3),
                        ),
                    )
                # out-proj of qtile 0 spread between qtile-1 attention blocks
                for g in range(GROUP):
                    fin = attn_block(1, g, fin)
                    outproj_iblocks(0, [2 * g, 2 * g + 1])
                fin()
                outproj_iblocks(1, list(range(8)))

    nc.finalize()
    return nc


def _get_nc():
    if "nc" not in _CACHE:
        _CACHE["nc"] = _build()
    return _CACHE["nc"]


def kernel(x, Wq, bq, Wk, bk, Wv, bv, Wo, bo):
    global LAST_RESULTS
    from concourse.bass_utils import run_bass_kernel_spmd

    bf = ml_dtypes.bfloat16
    x = np.asarray(x, np.float32)
    Wq = np.asarray(Wq, np.float32)
    Wk = np.asarray(Wk, np.float32)
    Wv = np.asarray(Wv, np.float32)
    Wo = np.asarray(Wo, np.float32)
    bq = np.asarray(bq, np.float32)
    bv = np.asarray(bv, np.float32)
    bo = np.asarray(bo, np.float32)

    nc = _get_nc()

    in_maps = []
    for c in range(8):
        b, h = divmod(c, NKV)
        xT = x[b].T  # [HIDDEN, S]
        xTh = np.ascontiguousarray(
            xT.reshape(NCH, 128, NBAND, BAND).transpose(2, 1, 0, 3)
        ).astype(bf)
        # wq[g]: [128, NCH, 128] per local head
        wqh = np.ascontiguousarray(
            Wq[:, h * 512:(h + 1) * 512]
            .reshape(NCH, 128, GROUP, 128).transpose(2, 1, 0, 3)
        ).astype(bf)
        wkh = np.ascontiguousarray(
            Wk[:, h * 128:(h + 1) * 128].reshape(NCH, 128, 128).transpose(1, 0, 2)
        ).astype(bf)
        wvh = np.ascontiguousarray(
            Wv[:, h * 128:(h + 1) * 128].reshape(NCH, 128, 128).transpose(1, 0, 2)
        ).astype(bf)
        woh = np.ascontiguousarray(
            Wo[h * 512:(h + 1) * 512, :].reshape(GROUP, 128, HIDDEN)
        ).astype(bf)
        bqh = np.ascontiguousarray(
            bq[h * 512:(h + 1) * 512].reshape(GROUP, 128).T
        )
        in_maps.append({
            "xT": xTh, "wq": wqh, "wk": wkh, "wv": wvh, "wo": woh,
            "bq": bqh,
        })

    res = run_bass_kernel_spmd(
        nc, in_maps, list(range(8)), trace=TRACE, tmpdir=TMPDIR
    )
    LAST_RESULTS = res

    # host-side constant bias: (bv repeated per head group) @ Wo + bo
    bv_rep = np.broadcast_to(
        bv.reshape(NKV, 1, D), (NKV, GROUP, D)
    ).reshape(HIDDEN)
    bias_row = bv_rep @ Wo + bo  # [HIDDEN]

    out = np.empty((B, S, HIDDEN), np.float32)
    for b in range(B):
        acc = res.results[b * NKV + 0]["out"].astype(np.float32)
        for h in range(1, NKV):
            acc = acc + res.results[b * NKV + h]["out"].astype(np.float32)
        out[b] = acc + bias_row
    return out
